# revision 20
# baseline (speedup 1.0000x reference)
"""Causal self-attention on 8 Trainium2 NeuronCores.

Problem: x[2,2048,2048] f32, W_qkv[2048,6144], W_out[2048,2048]
  qkv = x @ W_qkv; per-head causal softmax attention; out = attn @ W_out.

Sharding: core c handles batch b=c//4, head group hg=c%4 (4 of 16 heads).
Each core computes its heads' QKV projections, full causal attention for
those heads, and a partial output projection (its heads' rows of W_out).
Host sums the 4 partial outputs per batch. x is shipped pre-transposed
(xT[b] = x[b].T) so the device needs no transposes: every matmul wants the
contraction dim (D or Tk or Hd) on partitions.

Engine budget (per core): PE ~284us of matmul work is the floor
(fp8/DoubleRow would halve it but its ~5% per-element quantization error
carries through the zero-mean sums at full relative size — measured
6e-2, over the 2e-2 budget — so bf16 everywhere on the value path):
  - all matmul inputs bf16 (1 cy/row, half DMA/SBUF), f32 PSUM
  - weights resident in SBUF; startup DMAs chunk-interleaved (wq with
    xT-slab0, small groups first) so the first PSUM chain starts ~2us in
  - attention output lives in SBUF (no DRAM roundtrip)
  - softmax denominator: DVE f32 running sum of E tiles, reduced+
    broadcast across partitions in one GPSIMD partition_all_reduce per
    (head, query group) — no PE pass over E at all
  - diagonal blocks trimmed: fully-masked query columns are never
    computed (S/AV matmuls, exp); the remaining 128-wide triangle is
    masked with a DVE multiply by a precomputed triangle (built once on
    GPSIMD at init; keeping GPSIMD off the per-iteration critical path)
  - Phase B's key loop is software-pipelined: the AV matmul trails the
    S/exp emission by LAG iterations so its e-tile wait is satisfied
    when the in-order PE sequencer decodes it
  - Phase C (out-proj) micro-tasks are interleaved INTO Phase B's key
    loop on diagonal iterations (whose trimmed S matmuls are short but
    whose exp+mask latency is full); per-group normalize tails are
    deferred one group; y DMAs are per-512-column chunk to keep the
    end-of-kernel drain short
  - PSUM evictions split across ScalarE and DVE to balance their load
"""
import math

import numpy as np

import concourse.bass as bass
import concourse.bass_isa as bass_isa
import concourse.mybir as mybir
import concourse.tile as tile
from concourse import bacc
from concourse.bass_utils import run_bass_kernel_spmd

B, T, D = 2, 2048, 2048
H, Hd = 16, 128
N_CORES = 8
HL = 4            # heads per core
DL = HL * Hd      # 512: local hidden slice
P = 128
KC = D // P       # 16 contraction chunks of 128
NTB = T // P      # 16 row blocks of 128
QTW = 512         # query-group width
NQT = T // QTW    # 4 query groups
SCALE = 1.0 / math.sqrt(Hd)

f32 = mybir.dt.float32
f32r = mybir.dt.float32r
bf16 = mybir.dt.bfloat16
AF = mybir.ActivationFunctionType


def build_program(reps: int = 1, phases: str = "ABC"):
    nc = bacc.Bacc("TRN2", target_bir_lowering=False, debug=False,
                   num_devices=N_CORES)
    xT = nc.dram_tensor("xT", [D, T], bf16, kind="ExternalInput")
    wq = nc.dram_tensor("wq", [D, DL], bf16, kind="ExternalInput")
    wk = nc.dram_tensor("wk", [D, DL], bf16, kind="ExternalInput")
    wv = nc.dram_tensor("wv", [D, DL], bf16, kind="ExternalInput")
    wout = nc.dram_tensor("wout", [DL, D], bf16, kind="ExternalInput")
    y = nc.dram_tensor("y", [T, D], f32, kind="ExternalOutput")

    with tile.TileContext(nc) as tc:
        if reps > 1:
            with tc.For_i(0, reps, 1):
                _body(nc, tc, xT, wq, wk, wv, wout, y, phases)
        else:
            _body(nc, tc, xT, wq, wk, wv, wout, y, phases)
    nc.compile()
    return nc


def _body(nc, tc, xT, wq, wk, wv, wout, y, phases="ABC"):
    with (
        tc.tile_pool(name="persist", bufs=1) as persist,
    ):
        # triangular mask: tri[i, j] = 1.0 iff j >= i (for diagonal blocks)
        tri = persist.tile([P, P], bf16)
        with tc.tile_pool(name="init_scratch", bufs=1) as scratch:
            tri_f = scratch.tile([P, P], f32)
            nc.gpsimd.memset(tri_f[:], 1.0)
            nc.gpsimd.affine_select(
                out=tri_f[:], in_=tri_f[:],
                compare_op=mybir.AluOpType.is_ge,
                fill=0.0, base=0, channel_multiplier=-1,
                pattern=[[1, P]])
            nc.vector.tensor_copy(tri[:], tri_f[:])

        # resident weights, chunk-interleaved with the first xT slab below
        wq_sb = persist.tile([P, KC, DL], bf16)
        wk_sb = persist.tile([P, KC, DL], bf16)
        wv_sb = persist.tile([P, KC, DL], bf16)
        wout_sb = persist.tile([P, HL, D], bf16)

        with tc.tile_pool(name="qkv", bufs=1) as qkv_pool:
            qT_sb = qkv_pool.tile([P, HL, T], bf16)   # [Hd, h, Tq]
            kT_sb = qkv_pool.tile([P, HL, T], bf16)
            v_sb = qkv_pool.tile([P, NTB, DL], bf16)  # [Tk%128, kb, h*Hd]
            atT_sb = qkv_pool.tile([P, HL, T], bf16)  # attn outT [Hd, h, Tq]

            # ------------ Phase A: QKV projection ------------------------
            with (
                tc.tile_pool(name="a_xT", bufs=2) as xTpool,
                tc.tile_pool(name="ps_qk", bufs=4, space="PSUM") as ps_qk,
                tc.tile_pool(name="ps_v", bufs=4, space="PSUM") as ps_v,
            ):
                for s in range(NQT):  # 4 slabs of 512 T-cols
                    xTs = xTpool.tile([P, KC, QTW], bf16, tag="xT")
                    if s == 0:
                        # 512-row DMA groups: big enough to amortize the
                        # ~625ns HWDGE cost per DMA, small enough that the
                        # first matmuls start ~4us in. Round 1 (q) only
                        # needs wq + x; wk/wv/wout stream behind.
                        # (2,2,4,4,4) chunk groups: small first transfers
                        # so the first PSUM chain starts ~2us in
                        bounds = [0, 2, 4, 8, 12, 16]
                        for g in range(len(bounds) - 1):
                            r_sl = slice(bounds[g] * P, bounds[g + 1] * P)
                            k_sl = slice(bounds[g], bounds[g + 1])
                            nc.sync.dma_start(
                                wq_sb[:, k_sl, :],
                                wq.ap()[r_sl, :].rearrange(
                                    "(kc p) m -> p kc m", p=P))
                            nc.sync.dma_start(
                                xTs[:, k_sl, :],
                                xT.ap()[r_sl, s * QTW:(s + 1) * QTW]
                                .rearrange("(kc p) t -> p kc t", p=P))
                        G = 4
                        for g in range(KC // G):
                            r_sl = slice(g * G * P, (g + 1) * G * P)
                            k_sl = slice(g * G, (g + 1) * G)
                            nc.sync.dma_start(
                                wk_sb[:, k_sl, :],
                                wk.ap()[r_sl, :].rearrange(
                                    "(kc p) m -> p kc m", p=P))
                        for g in range(KC // G):
                            r_sl = slice(g * G * P, (g + 1) * G * P)
                            k_sl = slice(g * G, (g + 1) * G)
                            nc.sync.dma_start(
                                wv_sb[:, k_sl, :],
                                wv.ap()[r_sl, :].rearrange(
                                    "(kc p) m -> p kc m", p=P))
                        nc.sync.dma_start(
                            wout_sb[:],
                            wout.ap().rearrange("(hl p) d -> p hl d", p=P))
                        # slab 0: kc-outer rounds (all-q, all-k, v) so the
                        # PE chases the chunk DMA chain instead of stalling
                        # a whole PSUM group on the last chunk
                        for w_sb, dst in ((wq_sb, qT_sb), (wk_sb, kT_sb)):
                            pss = [ps_qk.tile([P, QTW], f32, tag="qk",
                                              name=f"qk_ps{i}")
                                   for i in range(HL)]
                            for kc in range(KC):
                                for h in range(HL):
                                    nc.tensor.matmul(
                                        pss[h][:],
                                        w_sb[:, kc, h * Hd:(h + 1) * Hd],
                                        xTs[:, kc, :],
                                        start=(kc == 0), stop=(kc == KC - 1))
                            for h in range(HL):
                                nc.vector.tensor_copy(
                                    dst[:, h, s * QTW:(s + 1) * QTW],
                                    pss[h][:])
                    else:
                        nc.sync.dma_start(
                            xTs[:],
                            xT.ap()[:, s * QTW:(s + 1) * QTW].rearrange(
                                "(kc p) t -> p kc t", p=P))
                        for h in range(HL):
                            for w_sb, dst in ((wq_sb, qT_sb), (wk_sb, kT_sb)):
                                ps = ps_qk.tile([P, QTW], f32, tag="qk",
                                                name="qk_ps")
                                for kc in range(KC):
                                    nc.tensor.matmul(
                                        ps[:],
                                        w_sb[:, kc, h * Hd:(h + 1) * Hd],
                                        xTs[:, kc, :],
                                        start=(kc == 0), stop=(kc == KC - 1))
                                nc.vector.tensor_copy(
                                    dst[:, h, s * QTW:(s + 1) * QTW], ps[:])
                    # v for all 4 heads (kc-outer; wv resident)
                    vps = [ps_v.tile([P, DL], f32, tag="v", name=f"vps{i}")
                           for i in range(4)]
                    for kc in range(KC):
                        for tsub in range(4):
                            nc.tensor.matmul(
                                vps[tsub][:],
                                xTs[:, kc, tsub * P:(tsub + 1) * P],
                                wv_sb[:, kc, :],
                                start=(kc == 0), stop=(kc == KC - 1))
                    for tsub in range(4):
                        nc.vector.tensor_copy(
                            v_sb[:, s * 4 + tsub, :], vps[tsub][:])

            # ------ Phases B+C fused ------------------------------------
            if "B" not in phases:
                return
            do_c = "C" in phases
            with (
                tc.tile_pool(name="b_e", bufs=6) as epool,
                tc.tile_pool(name="b_esum", bufs=2) as esumpool,
                tc.tile_pool(name="b_small", bufs=2) as bsmall,
                tc.tile_pool(name="c_y", bufs=2) as ypool,
                tc.tile_pool(name="ps_s", bufs=4, space="PSUM") as ps_s,
                tc.tile_pool(name="ps_o", bufs=2, space="PSUM") as ps_o,
                tc.tile_pool(name="ps_y", bufs=2, space="PSUM") as ps_y,
            ):
                y_sbs = {}

                def make_c_tasks(qt):
                    # 16 micro-tasks: one dc-chunk of one row-block each.
                    # Each is 4 PE matmuls (~850ns) + an eviction; the DMA
                    # fires after a row-block's 4 chunks are done.
                    tasks = []
                    for tb in range(qt * 4, qt * 4 + 4):
                        t_sl = slice(tb * P, (tb + 1) * P)
                        for dc in range(D // QTW):
                            def task(tb=tb, dc=dc, t_sl=t_sl):
                                if dc == 0:
                                    y_sbs[tb] = ypool.tile(
                                        [P, D], f32, tag="ysb",
                                        name="y_sb")
                                y_sb = y_sbs[tb]
                                y_ps = ps_y.tile([P, QTW], f32, tag="y",
                                                 name="y_ps")
                                for hh in range(HL):
                                    nc.tensor.matmul(
                                        y_ps[:],
                                        atT_sb[:, hh, t_sl],
                                        wout_sb[:, hh,
                                                dc * QTW:(dc + 1) * QTW],
                                        start=(hh == 0), stop=(hh == HL - 1))
                                d_sl = slice(dc * QTW, (dc + 1) * QTW)
                                if dc % 2 == 0:
                                    nc.scalar.copy(y_sb[:, d_sl], y_ps[:])
                                else:
                                    nc.vector.tensor_copy(
                                        y_sb[:, d_sl], y_ps[:])
                                # per-chunk DMA keeps the end-of-kernel
                                # tail to one 256KB transfer
                                nc.sync.dma_start(
                                    y.ap()[t_sl, d_sl], y_sb[:, d_sl])
                            tasks.append(task)
                    return tasks

                c_tasks = []        # pending out-proj micro-tasks
                pending_tail = []   # deferred normalize tail of prev group

                def emit_tail():
                    while pending_tail:
                        pending_tail.pop(0)()

                for qt in range(NQT):
                    nkb = (qt + 1) * 4
                    q_sl = slice(qt * QTW, (qt + 1) * QTW)
                    def pop_c(diag):
                        # diagonal iterations have short (trimmed) S
                        # matmuls but full exp+mask latency: slot the
                        # out-proj filler exactly there (16 per qt)
                        if not do_c or not diag or not c_tasks:
                            return
                        emit_tail()
                        c_tasks.pop(0)()

                    LAG = 3
                    for h in range(HL):
                        esum = esumpool.tile([P, QTW], f32r, tag="esum",
                                             name="esum")
                        o_ps = ps_o.tile([P, QTW], f32, tag="o", name="o_ps")
                        staged = []
                        for kb in range(nkb + LAG):
                            if kb < nkb:
                                m = kb - 4 * qt  # >=0 on diagonal blocks
                                off = max(m, 0) * P
                                s_ps = ps_s.tile([P, QTW], f32, tag="s",
                                                 name="s_ps")
                                nc.tensor.matmul(
                                    s_ps[:, off:],
                                    kT_sb[:, h, kb * P:(kb + 1) * P],
                                    qT_sb[:, h,
                                          qt * QTW + off:(qt + 1) * QTW],
                                    start=True, stop=True)
                                e_sb = epool.tile([P, QTW], bf16, tag="e",
                                                  name="e_sb")
                                nc.scalar.activation(
                                    e_sb[:, off:], s_ps[:, off:], AF.Exp,
                                    scale=float(SCALE))
                                if m >= 0:
                                    # triangular mask on the 128-wide
                                    # diagonal chunk: keep iff q >= key
                                    # (DVE mul; GPSIMD dispatch is too
                                    # slow for the per-iteration path)
                                    nc.vector.tensor_mul(
                                        e_sb[:, off:off + P],
                                        e_sb[:, off:off + P], tri[:])
                                if kb == 0:
                                    nc.vector.tensor_copy(esum[:], e_sb[:])
                                else:
                                    nc.vector.tensor_add(
                                        esum[:, off:], e_sb[:, off:],
                                        esum[:, off:])
                                staged.append((kb, off, e_sb))
                            if kb == 2:
                                emit_tail()
                            if kb >= LAG:
                                # AV trails S/exp by LAG iterations so the
                                # in-order PE sequencer never decodes an
                                # unsatisfied e-tile wait
                                j, joff, je = staged.pop(0)
                                pop_c(j - 4 * qt >= 0)
                                nc.tensor.matmul(
                                    o_ps[:, joff:],
                                    v_sb[:, j, h * Hd:(h + 1) * Hd],
                                    je[:, joff:],
                                    start=(j == 0), stop=(j == nkb - 1))

                        def tail(h=h, qt=qt, q_sl=q_sl, esum=esum,
                                 o_ps=o_ps):
                            # denominator: all-reduce esum across partitions
                            # on GPSIMD (result broadcast to all partitions)
                            d_bc = bsmall.tile([P, QTW], f32r, tag="dbc",
                                               name="d_bc")
                            nc.gpsimd.partition_all_reduce(
                                d_bc[:], esum[:], channels=P,
                                reduce_op=bass_isa.ReduceOp.add)
                            o_raw = bsmall.tile([P, QTW], bf16, tag="oraw",
                                                name="o_raw")
                            nc.scalar.copy(o_raw[:], o_ps[:])
                            rec = bsmall.tile([P, QTW], f32r, tag="rec",
                                              name="rec")
                            with nc.allow_low_precision(
                                    reason="f32r reciprocal, 2^-19 rel"):
                                nc.vector.reciprocal(rec[:], d_bc[:])
                            nc.vector.tensor_mul(
                                atT_sb[:, h, q_sl], o_raw[:], rec[:])
                        emit_tail()  # previous group's tail, if still queued
                        pending_tail.append(tail)
                    if do_c:
                        # queue out-proj for this query group; emitted
                        # during the next group's key loop
                        c_tasks.extend(make_c_tasks(qt))
                emit_tail()
                while c_tasks:
                    c_tasks.pop(0)()


def prepare_in_maps(x, W_qkv, W_out):
    bfdt = mybir.dt.np(bf16)
    x = np.asarray(x, dtype=np.float32)
    W_qkv = np.asarray(W_qkv, dtype=np.float32)
    W_out = np.asarray(W_out, dtype=np.float32)
    Wr = W_qkv.reshape(D, 3, H, Hd)
    Wo = W_out.reshape(H, Hd, D)
    xTs = [np.ascontiguousarray(x[b].T).astype(bfdt) for b in range(B)]
    in_maps = []
    for c in range(N_CORES):
        b, hg = c // 4, c % 4
        hs = slice(hg * HL, (hg + 1) * HL)
        in_maps.append({
            "xT": xTs[b],
            "wq": np.ascontiguousarray(
                Wr[:, 0, hs, :].reshape(D, DL)).astype(bfdt),
            "wk": np.ascontiguousarray(
                Wr[:, 1, hs, :].reshape(D, DL)).astype(bfdt),
            "wv": np.ascontiguousarray(
                Wr[:, 2, hs, :].reshape(D, DL)).astype(bfdt),
            "wout": np.ascontiguousarray(
                Wo[hs].reshape(DL, D)).astype(bfdt),
        })
    return in_maps


def combine_outputs(results):
    out = np.zeros((B, T, D), dtype=np.float32)
    for c in range(N_CORES):
        out[c // 4] += results[c]["y"]
    return out


_PROGRAM_CACHE = {}


def kernel(x, W_qkv, W_out):
    in_maps = prepare_in_maps(x, W_qkv, W_out)
    if 1 not in _PROGRAM_CACHE:
        _PROGRAM_CACHE[1] = build_program(1)
    nc = _PROGRAM_CACHE[1]
    res = run_bass_kernel_spmd(nc, in_maps, core_ids=list(range(N_CORES)))
    return combine_outputs(res.results)


# revision 23
# speedup vs baseline: 1.1551x; 1.1551x over previous
"""Causal self-attention on 8 Trainium2 NeuronCores.

Problem: x[2,2048,2048] f32, W_qkv[2048,6144], W_out[2048,2048]
  qkv = x @ W_qkv; per-head causal softmax attention; out = attn @ W_out.

Sharding: core c handles batch b=c//4, head group hg=c%4 (4 of 16 heads).
Each core computes its heads' QKV projections, full causal attention for
those heads, and a partial output projection (its heads' rows of W_out).
Host sums the 4 partial outputs per batch. x is shipped pre-transposed
(xT[b] = x[b].T) so the device needs no transposes: every matmul wants the
contraction dim (D or Tk or Hd) on partitions.

Engine budget (per core): PE ~284us of matmul work is the floor
(fp8/DoubleRow would halve it but its ~5% per-element quantization error
carries through the zero-mean sums at full relative size — measured
6e-2, over the 2e-2 budget — so bf16 everywhere on the value path):
  - all matmul inputs bf16 (1 cy/row, half DMA/SBUF), f32 PSUM
  - weights resident in SBUF; startup DMAs chunk-interleaved (wq with
    xT-slab0, small groups first) so the first PSUM chain starts ~2us in
  - attention output lives in SBUF (no DRAM roundtrip)
  - softmax denominator: DVE f32 running sum of E tiles, reduced+
    broadcast across partitions in one GPSIMD partition_all_reduce per
    (head, query group) — no PE pass over E at all
  - diagonal blocks trimmed: fully-masked query columns are never
    computed (S/AV matmuls, exp); the remaining 128-wide triangle is
    masked in place on GPSIMD via affine_select
  - Phase C (out-proj) micro-tasks are interleaved INTO Phase B's key
    loop on diagonal iterations (whose trimmed S matmuls are short but
    whose exp+mask latency is full); per-group normalize tails are
    deferred one group; y DMAs are per-512-column chunk to keep the
    end-of-kernel drain short
  - PSUM evictions split across ScalarE and DVE to balance their load
"""
import math

import numpy as np

import concourse.bass as bass
import concourse.bass_isa as bass_isa
import concourse.mybir as mybir
import concourse.tile as tile
from concourse import bacc
from concourse.bass_utils import run_bass_kernel_spmd

B, T, D = 2, 2048, 2048
H, Hd = 16, 128
N_CORES = 8
HL = 4            # heads per core
DL = HL * Hd      # 512: local hidden slice
P = 128
KC = D // P       # 16 contraction chunks of 128
NTB = T // P      # 16 row blocks of 128
QTW = 512         # query-group width
NQT = T // QTW    # 4 query groups
SCALE = 1.0 / math.sqrt(Hd)

f32 = mybir.dt.float32
f32r = mybir.dt.float32r
bf16 = mybir.dt.bfloat16
AF = mybir.ActivationFunctionType


def build_program(reps: int = 1, phases: str = "ABC"):
    nc = bacc.Bacc("TRN2", target_bir_lowering=False, debug=False,
                   num_devices=N_CORES)
    xT = nc.dram_tensor("xT", [D, T], bf16, kind="ExternalInput")
    wq = nc.dram_tensor("wq", [D, DL], bf16, kind="ExternalInput")
    wk = nc.dram_tensor("wk", [D, DL], bf16, kind="ExternalInput")
    wv = nc.dram_tensor("wv", [D, DL], bf16, kind="ExternalInput")
    wout = nc.dram_tensor("wout", [DL, D], bf16, kind="ExternalInput")
    y = nc.dram_tensor("y", [T, D], f32, kind="ExternalOutput")

    with tile.TileContext(nc) as tc:
        if reps > 1:
            with tc.For_i(0, reps, 1):
                _body(nc, tc, xT, wq, wk, wv, wout, y, phases)
        else:
            _body(nc, tc, xT, wq, wk, wv, wout, y, phases)
    nc.compile()
    return nc


def _body(nc, tc, xT, wq, wk, wv, wout, y, phases="ABC"):
    with (
        tc.tile_pool(name="persist", bufs=1) as persist,
    ):
        # triangular mask: tri[i, j] = 1.0 iff j >= i (for diagonal blocks)
        tri = persist.tile([P, P], bf16)
        with tc.tile_pool(name="init_scratch", bufs=1) as scratch:
            tri_f = scratch.tile([P, P], f32)
            nc.gpsimd.memset(tri_f[:], 1.0)
            nc.gpsimd.affine_select(
                out=tri_f[:], in_=tri_f[:],
                compare_op=mybir.AluOpType.is_ge,
                fill=0.0, base=0, channel_multiplier=-1,
                pattern=[[1, P]])
            nc.vector.tensor_copy(tri[:], tri_f[:])

        # resident weights, chunk-interleaved with the first xT slab below
        wq_sb = persist.tile([P, KC, DL], bf16)
        wk_sb = persist.tile([P, KC, DL], bf16)
        wv_sb = persist.tile([P, KC, DL], bf16)
        wout_sb = persist.tile([P, HL, D], bf16)

        with tc.tile_pool(name="qkv", bufs=1) as qkv_pool:
            qT_sb = qkv_pool.tile([P, HL, T], bf16)   # [Hd, h, Tq]
            kT_sb = qkv_pool.tile([P, HL, T], bf16)
            v_sb = qkv_pool.tile([P, NTB, DL], bf16)  # [Tk%128, kb, h*Hd]
            atT_sb = qkv_pool.tile([P, HL, T], bf16)  # attn outT [Hd, h, Tq]

            # ------------ Phase A: QKV projection ------------------------
            with (
                tc.tile_pool(name="a_xT", bufs=2) as xTpool,
                tc.tile_pool(name="ps_qk", bufs=4, space="PSUM") as ps_qk,
                tc.tile_pool(name="ps_v", bufs=4, space="PSUM") as ps_v,
            ):
                for s in range(NQT):  # 4 slabs of 512 T-cols
                    xTs = xTpool.tile([P, KC, QTW], bf16, tag="xT")
                    if s == 0:
                        # 512-row DMA groups: big enough to amortize the
                        # ~625ns HWDGE cost per DMA, small enough that the
                        # first matmuls start ~4us in. Round 1 (q) only
                        # needs wq + x; wk/wv/wout stream behind.
                        # (2,2,4,4,4) chunk groups: small first transfers
                        # so the first PSUM chain starts ~2us in
                        bounds = [0, 2, 4, 8, 12, 16]
                        for g in range(len(bounds) - 1):
                            r_sl = slice(bounds[g] * P, bounds[g + 1] * P)
                            k_sl = slice(bounds[g], bounds[g + 1])
                            nc.sync.dma_start(
                                wq_sb[:, k_sl, :],
                                wq.ap()[r_sl, :].rearrange(
                                    "(kc p) m -> p kc m", p=P))
                            nc.sync.dma_start(
                                xTs[:, k_sl, :],
                                xT.ap()[r_sl, s * QTW:(s + 1) * QTW]
                                .rearrange("(kc p) t -> p kc t", p=P))
                        G = 4
                        for g in range(KC // G):
                            r_sl = slice(g * G * P, (g + 1) * G * P)
                            k_sl = slice(g * G, (g + 1) * G)
                            nc.sync.dma_start(
                                wk_sb[:, k_sl, :],
                                wk.ap()[r_sl, :].rearrange(
                                    "(kc p) m -> p kc m", p=P))
                        for g in range(KC // G):
                            r_sl = slice(g * G * P, (g + 1) * G * P)
                            k_sl = slice(g * G, (g + 1) * G)
                            nc.sync.dma_start(
                                wv_sb[:, k_sl, :],
                                wv.ap()[r_sl, :].rearrange(
                                    "(kc p) m -> p kc m", p=P))
                        nc.sync.dma_start(
                            wout_sb[:],
                            wout.ap().rearrange("(hl p) d -> p hl d", p=P))
                        # slab 0: kc-outer rounds (all-q, all-k, v) so the
                        # PE chases the chunk DMA chain instead of stalling
                        # a whole PSUM group on the last chunk
                        for w_sb, dst in ((wq_sb, qT_sb), (wk_sb, kT_sb)):
                            pss = [ps_qk.tile([P, QTW], f32, tag="qk",
                                              name=f"qk_ps{i}")
                                   for i in range(HL)]
                            for kc in range(KC):
                                for h in range(HL):
                                    nc.tensor.matmul(
                                        pss[h][:],
                                        w_sb[:, kc, h * Hd:(h + 1) * Hd],
                                        xTs[:, kc, :],
                                        start=(kc == 0), stop=(kc == KC - 1))
                            for h in range(HL):
                                nc.vector.tensor_copy(
                                    dst[:, h, s * QTW:(s + 1) * QTW],
                                    pss[h][:])
                    else:
                        nc.sync.dma_start(
                            xTs[:],
                            xT.ap()[:, s * QTW:(s + 1) * QTW].rearrange(
                                "(kc p) t -> p kc t", p=P))
                        for h in range(HL):
                            for w_sb, dst in ((wq_sb, qT_sb), (wk_sb, kT_sb)):
                                ps = ps_qk.tile([P, QTW], f32, tag="qk",
                                                name="qk_ps")
                                for kc in range(KC):
                                    nc.tensor.matmul(
                                        ps[:],
                                        w_sb[:, kc, h * Hd:(h + 1) * Hd],
                                        xTs[:, kc, :],
                                        start=(kc == 0), stop=(kc == KC - 1))
                                nc.vector.tensor_copy(
                                    dst[:, h, s * QTW:(s + 1) * QTW], ps[:])
                    # v for all 4 heads (kc-outer; wv resident)
                    vps = [ps_v.tile([P, DL], f32, tag="v", name=f"vps{i}")
                           for i in range(4)]
                    for kc in range(KC):
                        for tsub in range(4):
                            nc.tensor.matmul(
                                vps[tsub][:],
                                xTs[:, kc, tsub * P:(tsub + 1) * P],
                                wv_sb[:, kc, :],
                                start=(kc == 0), stop=(kc == KC - 1))
                    for tsub in range(4):
                        nc.vector.tensor_copy(
                            v_sb[:, s * 4 + tsub, :], vps[tsub][:])

            # ------ Phases B+C fused ------------------------------------
            if "B" not in phases:
                return
            do_c = "C" in phases
            with (
                tc.tile_pool(name="b_e", bufs=9) as epool,
                tc.tile_pool(name="b_esum", bufs=3) as esumpool,
                tc.tile_pool(name="b_small", bufs=3) as bsmall,
                tc.tile_pool(name="c_y", bufs=3) as ypool,
                tc.tile_pool(name="ps_s", bufs=4, space="PSUM") as ps_s,
                tc.tile_pool(name="ps_o", bufs=2, space="PSUM") as ps_o,
                tc.tile_pool(name="ps_y", bufs=2, space="PSUM") as ps_y,
            ):
                y_sbs = {}

                def make_c_tasks(qt):
                    # 16 micro-tasks: one dc-chunk of one row-block each.
                    # Each is 4 PE matmuls (~850ns) + an eviction; the DMA
                    # fires after a row-block's 4 chunks are done.
                    tasks = []
                    for tb in range(qt * 4, qt * 4 + 4):
                        t_sl = slice(tb * P, (tb + 1) * P)
                        for dc in range(D // QTW):
                            def task(tb=tb, dc=dc, t_sl=t_sl):
                                if dc == 0:
                                    y_sbs[tb] = ypool.tile(
                                        [P, D], f32, tag="ysb",
                                        name="y_sb")
                                y_sb = y_sbs[tb]
                                y_ps = ps_y.tile([P, QTW], f32, tag="y",
                                                 name="y_ps")
                                for hh in range(HL):
                                    nc.tensor.matmul(
                                        y_ps[:],
                                        atT_sb[:, hh, t_sl],
                                        wout_sb[:, hh,
                                                dc * QTW:(dc + 1) * QTW],
                                        start=(hh == 0), stop=(hh == HL - 1))
                                d_sl = slice(dc * QTW, (dc + 1) * QTW)
                                if dc % 2 == 0:
                                    nc.scalar.copy(y_sb[:, d_sl], y_ps[:])
                                else:
                                    nc.vector.tensor_copy(
                                        y_sb[:, d_sl], y_ps[:])
                                # per-chunk DMA keeps the end-of-kernel
                                # tail to one 256KB transfer
                                nc.sync.dma_start(
                                    y.ap()[t_sl, d_sl], y_sb[:, d_sl])
                            tasks.append(task)
                    return tasks

                c_tasks = []        # pending out-proj micro-tasks
                pending_tail = []   # deferred normalize tail of prev group

                def emit_tail():
                    while pending_tail:
                        pending_tail.pop(0)()

                for qt in range(NQT):
                    nkb = (qt + 1) * 4
                    q_sl = slice(qt * QTW, (qt + 1) * QTW)
                    def pop_c(diag):
                        # diagonal iterations have short (trimmed) S
                        # matmuls but full exp+mask latency: slot the
                        # out-proj filler exactly there (16 per qt)
                        if not do_c or not diag or not c_tasks:
                            return
                        emit_tail()
                        c_tasks.pop(0)()

                    for h in range(HL):
                        esum = esumpool.tile([P, QTW], f32r, tag="esum",
                                             name="esum")
                        o_ps = ps_o.tile([P, QTW], f32, tag="o", name="o_ps")
                        for kb in range(nkb):
                            m = kb - 4 * qt  # >=0 on diagonal blocks
                            off = max(m, 0) * P
                            s_ps = ps_s.tile([P, QTW], f32, tag="s",
                                             name="s_ps")
                            nc.tensor.matmul(
                                s_ps[:, off:],
                                kT_sb[:, h, kb * P:(kb + 1) * P],
                                qT_sb[:, h, qt * QTW + off:(qt + 1) * QTW],
                                start=True, stop=True)
                            e_sb = epool.tile([P, QTW], bf16, tag="e",
                                              name="e_sb")
                            nc.scalar.activation(
                                e_sb[:, off:], s_ps[:, off:], AF.Exp,
                                scale=float(SCALE))
                            if m >= 0:
                                # triangular mask on the 128-wide diagonal
                                # chunk: keep iff q-col >= key-row
                                nc.gpsimd.affine_select(
                                    out=e_sb[:, off:off + P],
                                    in_=e_sb[:, off:off + P],
                                    compare_op=mybir.AluOpType.is_ge,
                                    fill=0.0, base=0, channel_multiplier=-1,
                                    pattern=[[1, P]])
                            if kb == 0:
                                nc.vector.tensor_copy(esum[:], e_sb[:])
                            else:
                                nc.vector.tensor_add(
                                    esum[:, off:], e_sb[:, off:],
                                    esum[:, off:])
                            # feed the PE queue while exp runs
                            if kb == 1:
                                emit_tail()
                            pop_c(m >= 0)
                            nc.tensor.matmul(
                                o_ps[:, off:],
                                v_sb[:, kb, h * Hd:(h + 1) * Hd],
                                e_sb[:, off:],
                                start=(kb == 0), stop=(kb == nkb - 1))

                        def tail(h=h, qt=qt, q_sl=q_sl, esum=esum,
                                 o_ps=o_ps):
                            # denominator: all-reduce esum across partitions
                            # on GPSIMD (result broadcast to all partitions)
                            d_bc = bsmall.tile([P, QTW], f32r, tag="dbc",
                                               name="d_bc")
                            nc.gpsimd.partition_all_reduce(
                                d_bc[:], esum[:], channels=P,
                                reduce_op=bass_isa.ReduceOp.add)
                            o_raw = bsmall.tile([P, QTW], bf16, tag="oraw",
                                                name="o_raw")
                            nc.scalar.copy(o_raw[:], o_ps[:])
                            rec = bsmall.tile([P, QTW], f32r, tag="rec",
                                              name="rec")
                            with nc.allow_low_precision(
                                    reason="f32r reciprocal, 2^-19 rel"):
                                nc.vector.reciprocal(rec[:], d_bc[:])
                            nc.vector.tensor_mul(
                                atT_sb[:, h, q_sl], o_raw[:], rec[:])
                        emit_tail()  # previous group's tail, if still queued
                        pending_tail.append(tail)
                    if do_c:
                        # queue out-proj for this query group; emitted
                        # during the next group's key loop
                        c_tasks.extend(make_c_tasks(qt))
                emit_tail()
                while c_tasks:
                    c_tasks.pop(0)()


def prepare_in_maps(x, W_qkv, W_out):
    bfdt = mybir.dt.np(bf16)
    x = np.asarray(x, dtype=np.float32)
    W_qkv = np.asarray(W_qkv, dtype=np.float32)
    W_out = np.asarray(W_out, dtype=np.float32)
    Wr = W_qkv.reshape(D, 3, H, Hd)
    Wo = W_out.reshape(H, Hd, D)
    xTs = [np.ascontiguousarray(x[b].T).astype(bfdt) for b in range(B)]
    in_maps = []
    for c in range(N_CORES):
        b, hg = c // 4, c % 4
        hs = slice(hg * HL, (hg + 1) * HL)
        in_maps.append({
            "xT": xTs[b],
            "wq": np.ascontiguousarray(
                Wr[:, 0, hs, :].reshape(D, DL)).astype(bfdt),
            "wk": np.ascontiguousarray(
                Wr[:, 1, hs, :].reshape(D, DL)).astype(bfdt),
            "wv": np.ascontiguousarray(
                Wr[:, 2, hs, :].reshape(D, DL)).astype(bfdt),
            "wout": np.ascontiguousarray(
                Wo[hs].reshape(DL, D)).astype(bfdt),
        })
    return in_maps


def combine_outputs(results):
    out = np.zeros((B, T, D), dtype=np.float32)
    for c in range(N_CORES):
        out[c // 4] += results[c]["y"]
    return out


_PROGRAM_CACHE = {}


def kernel(x, W_qkv, W_out):
    in_maps = prepare_in_maps(x, W_qkv, W_out)
    if 1 not in _PROGRAM_CACHE:
        _PROGRAM_CACHE[1] = build_program(1)
    nc = _PROGRAM_CACHE[1]
    res = run_bass_kernel_spmd(nc, in_maps, core_ids=list(range(N_CORES)))
    return combine_outputs(res.results)
